# revision 1
# baseline (speedup 1.0000x reference)
"""DimeNet-diabat Trainium2 kernel: 8-core SPMD Bass implementation.

Sharding: edges/angles/atoms partitioned by owner atom core (atom a -> core
a // (NA/8)); molecules never straddle cores. Parameters replicated.

Device pipeline (per core, identical SPMD program):
  - Edge MLP chain feature-major ([128 feat partitions, edges free]).
  - Angle message passing: dma_gather of local x_kj_e rows, multiply by
    host-precomputed sbf_p, AllToAll products to ji-owner cores, then
    dma_scatter_add into per-edge aggregates in duplicate-free waves.
  - Out-blocks: per-atom segment sums as PE matmuls (transposed-m chunks
    against a static e_rbf-scaled indicator "S6"), atoms rebalanced into
    edge-count-balanced 21-atom windows so the schedule is static across
    cores; dense heads on local atoms.
Host: index relabeling, basis functions, embedding gather, molecule sums,
final 2x2 eigendecomposition.
"""

import os
import ml_dtypes
import numpy as np

# ---------------- problem constants (hardcoded from spec) ----------------
CUTOFF = 5.0
ENV_P = 6
N_RBF, N_SPHER, L_SPHER = 6, 6, 7
SBF = N_SPHER * L_SPHER
EMB, INT_DIM, BEMB = 128, 64, 8
N_CONV, N_KEYS = 4, 3
NLEVEL = N_CONV + 1

FULL_CFG = dict(NA=8000, NE=200000, NW=600000, NG=80, APM=100)

NCORES = 8
P = 128
WA = 43                 # atoms per window (6*43=258 free dim, f32r-fast)
TILE = 1024             # edge macro-tile (2 PSUM banks per activation span)
GCH = 4096              # gather/product chunk (slots)
SCAT_BSZ = 512          # scatter piece rows per src core


# ============================ host preprocessing ============================

def _envelope(x):
    p = ENV_P
    a = -(p + 1) * (p + 2) / 2.0
    b = float(p * (p + 2))
    c = -p * (p + 1) / 2.0
    with np.errstate(divide="ignore"):
        env = 1.0 / x + a * x ** (p - 1) + b * x ** p + c * x ** (p + 1)
    return np.where(x < 1.0, env, 0.0).astype(np.float32)


def _wrap16(idx):
    """int16 index list -> [128, ceil(n/16)] wrapped (w -> [w%16, w//16]),
    replicated across the 8 Q7 cores."""
    n = len(idx)
    cols = -(-n // 16)
    flat = np.zeros(cols * 16, np.int16)
    flat[:n] = np.asarray(idx, np.int16)
    buf = flat.reshape(cols, 16).T.copy()
    return np.tile(buf, (8, 1)).copy()


def _roundup(x, m):
    return int(-(-x // m) * m)


def host_prep(inputs, cfg):
    NA, NE, NW = cfg["NA"], cfg["NE"], cfg["NW"]
    NG, APM = cfg["NG"], cfg["APM"]
    C = NCORES
    APC = NA // C
    NWIN = -(-APC // WA)
    assert NA % C == 0 and APC % APM == 0

    f32 = np.float32
    xyz = np.asarray(inputs["xyz"], f32)
    nbr = np.asarray(inputs["nbr_list"], np.int64)
    ang_l = np.asarray(inputs["angle_list"], np.int64)
    kj_idx = np.asarray(inputs["kj_idx"], np.int64)
    ji_idx = np.asarray(inputs["ji_idx"], np.int64)
    z = np.asarray(inputs["z"], np.int64)

    # ---- geometry / basis ----
    d = np.linalg.norm(xyz[nbr[:, 0]] - xyz[nbr[:, 1]], axis=-1).astype(f32)
    xs = d / f32(CUTOFF)
    n_ar = np.arange(1, N_RBF + 1, dtype=f32)
    e_rbf = (_envelope(xs)[:, None]
             * np.sin(np.pi * n_ar[None, :] * xs[:, None])).astype(f32)

    r_ji = xyz[ang_l[:, 0]] - xyz[ang_l[:, 1]]
    r_jk = xyz[ang_l[:, 2]] - xyz[ang_l[:, 1]]
    cos_t = np.sum(r_ji * r_jk, axis=-1)
    cr = np.cross(r_ji, r_jk)
    sin_t = np.sqrt(np.sum(cr * cr, axis=-1) + 1e-12)
    alpha = np.arctan2(sin_t, cos_t).astype(f32)
    x_kj = xs[kj_idx]
    ns = np.arange(1, N_SPHER + 1, dtype=f32)
    rad = _envelope(x_kj)[:, None] * np.sin(np.pi * ns[None, :] * x_kj[:, None])
    ls = np.arange(L_SPHER, dtype=f32)
    ang_b = np.cos(ls[None, :] * alpha[:, None])
    a_sbf = (ang_b[:, :, None] * rad[:, None, :]).reshape(NW, SBF).astype(f32)

    # ---- embedding gather (host) ----
    emb_z = np.asarray(inputs["emb_z"], f32)
    emb_w = np.asarray(inputs["emb_w"], f32)
    h = emb_z[z]
    hja = (h[nbr[:, 1]] @ emb_w[:EMB]
           + h[nbr[:, 0]] @ emb_w[EMB:2 * EMB]).astype(f32)

    # ---- atom window balancing (per core) ----
    i_atom = nbr[:, 0]
    deg = np.bincount(i_atom, minlength=NA)
    # window id + position for every atom; greedy LPT bin packing per core
    win_of_atom = np.empty(NA, np.int64)
    slot_of_atom = np.empty(NA, np.int64)   # position within window (0..WA-1)
    budgets = np.zeros((C, NWIN), np.int64)
    for q in range(C):
        a0 = q * APC
        order = np.argsort(-deg[a0:a0 + APC], kind="stable")
        fill = np.zeros(NWIN, np.int64)
        cnt = np.zeros(NWIN, np.int64)
        for a in order:
            cand = np.flatnonzero(cnt < WA)
            w = cand[np.argmin(fill[cand])]
            win_of_atom[a0 + a] = w
            slot_of_atom[a0 + a] = cnt[w]
            fill[w] += deg[a0 + a]
            cnt[w] += 1
        budgets[q] = fill
    budget_w = budgets.max(axis=0)          # shared static budgets [NWIN]
    wstart = np.zeros(NWIN + 1, np.int64)
    wstart[1:] = np.cumsum(budget_w)
    EPAD = _roundup(int(wstart[-1]), TILE)
    APAD = NWIN * WA

    # ---- edge relabeling: per core, window-major then atom ----
    owner = i_atom // APC
    ekey = (owner * NWIN + win_of_atom[i_atom]) * NA + i_atom
    order_e = np.argsort(ekey, kind="stable")
    # position of each sorted edge inside its (core, window) group
    ow_sorted = owner[order_e]
    win_sorted = win_of_atom[i_atom[order_e]]
    gk = ow_sorted * NWIN + win_sorted
    gchg = np.r_[0, np.flatnonzero(np.diff(gk)) + 1]
    glen = np.diff(np.r_[gchg, NE])
    posw = np.arange(NE) - np.repeat(gchg, glen)
    l_new_sorted = wstart[win_sorted] + posw          # local padded edge id
    l_of_old = np.empty(NE, np.int64)
    l_of_old[order_e] = l_new_sorted
    q_of_old = owner

    # per-core local edge arrays (padded layout)
    e_rbf_c = np.zeros((C, EPAD, N_RBF), f32)
    hja_c = np.zeros((C, EPAD, EMB), f32)
    awin_c = np.full((C, EPAD, 2), -1, np.int64)      # (window, slot) per edge
    for q in range(C):
        sel = order_e[q_of_old[order_e] == q]
        li = l_of_old[sel]
        e_rbf_c[q, li] = e_rbf[sel]
        hja_c[q, li] = hja[sel]
        ia = i_atom[sel]
        awin_c[q, li, 0] = win_of_atom[ia]
        awin_c[q, li, 1] = slot_of_atom[ia]

    # ---- angle bookkeeping ----
    ji_new_q = q_of_old[ji_idx]
    kj_new_q = q_of_old[kj_idx]
    l_ji = l_of_old[ji_idx]
    l_kj = l_of_old[kj_idx]
    dst, src = ji_new_q, kj_new_q

    # wave = rank within the ji run, in randomized order (balances the
    # per-(src,dst,wave) counts across srcs)
    rng_ = np.random.default_rng(12345)
    rnd = rng_.permutation(NW)
    ow2 = np.lexsort((rnd, dst * EPAD + l_ji))
    jid = (dst * EPAD + l_ji)[ow2]
    runchg = np.r_[0, np.flatnonzero(np.diff(jid)) + 1]
    runlen = np.diff(np.r_[runchg, NW])
    wave = np.empty(NW, np.int64)
    wave[ow2] = np.arange(NW) - np.repeat(runchg, runlen)
    NWAVES = int(wave.max()) + 1

    og = np.lexsort((l_kj, wave, dst, src))
    cnt = np.zeros((C, C, NWAVES), np.int64)
    np.add.at(cnt, (src, dst, wave), 1)
    B_k = (-(-cnt.max(axis=(0, 1)) // 128) * 128).astype(np.int64)
    cumB = np.zeros(NWAVES + 1, np.int64)
    cumB[1:] = np.cumsum(B_k)
    BMAX = _roundup(int(cumB[-1]), 128)
    SLOTS = _roundup(C * BMAX, GCH)

    gkey = (src[og] * C + dst[og]) * NWAVES + wave[og]
    gchg2 = np.r_[0, np.flatnonzero(np.diff(gkey)) + 1]
    glen2 = np.diff(np.r_[gchg2, NW])
    pos = np.arange(NW) - np.repeat(gchg2, glen2)
    slot = dst[og] * BMAX + cumB[wave[og]] + pos      # sender slot of angle

    # sender-side arrays: gather idx + sbf_p in slot order
    iw_sbf1 = np.asarray(inputs["iw_sbf1"], f32)
    iw_sbf2 = np.asarray(inputs["iw_sbf2"], f32)
    wc = np.einsum("csb,cbi->csi", iw_sbf1, iw_sbf2)
    gidx_c = np.zeros((C, SLOTS), np.int16)
    sbfp_c = np.zeros((C, N_CONV, SLOTS, INT_DIM), ml_dtypes.bfloat16)
    for q in range(C):
        m = src[og] == q
        sl = slot[m]
        gidx_c[q, sl] = l_kj[og][m].astype(np.int16)
        sp = np.einsum("ws,csi->cwi", a_sbf[og[m]], wc)       # [4, n, 64]
        sbfp_c[q][:, sl, :] = sp.astype(ml_dtypes.bfloat16)

    # receiver scatter targets per (src, wave, pos); pads -> scratch row EPAD
    maxB = int(B_k.max()) if NWAVES else 1
    scat_tgt = np.full((C, C, NWAVES, maxB), EPAD, np.int64)
    mq = dst[og]
    scat_tgt[mq, src[og], wave[og], pos] = l_ji[og]

    pieces = []
    for k in range(NWAVES):
        b = 0
        while b < B_k[k]:
            bsz = int(min(SCAT_BSZ, B_k[k] - b))
            pieces.append((k, b, bsz))
            b += bsz
    scat_idx_c = []
    for q in range(C):
        cols = []
        for (k, b0, bsz) in pieces:
            tg = scat_tgt[q, :, k, b0:b0 + bsz].reshape(-1)   # (s-major, pos)
            cols.append(_wrap16(tg))
        scat_idx_c.append(np.concatenate(cols, axis=1))
    scat_idx_c = np.stack(scat_idx_c)
    gidx_w_c = np.stack([_wrap16(gidx_c[q]) for q in range(C)])

    # ---- S6 static indicator + schedule (shared structure) ----
    nchunk = EPAD // P
    # instances: chunk x windows it can touch (static, from budgets)
    insts = []                    # (chunk, window)
    sched = [[] for _ in range(NWIN)]
    for ch in range(nchunk):
        lo, hi = ch * P, ch * P + P - 1
        w0 = int(np.searchsorted(wstart[1:], lo, side="right"))
        w1 = int(np.searchsorted(wstart[1:], hi, side="right"))
        w1 = min(w1, NWIN - 1)
        w0 = min(w0, NWIN - 1)
        for w in range(w0, w1 + 1):
            sched[w].append((len(insts), ch))
            insts.append((ch, w))
    NINST = len(insts)
    S6_c = np.zeros((C, NINST, P, 6 * WA), f32)
    for q in range(C):
        for idx, (ch, w) in enumerate(insts):
            rows = np.arange(ch * P, ch * P + P)
            wn = awin_c[q, rows, 0]
            sl = awin_c[q, rows, 1]
            sel = np.flatnonzero(wn == w)
            if len(sel) == 0:
                continue
            for r in range(6):
                S6_c[q, idx, sel, r * WA + sl[sel]] = \
                    e_rbf_c[q][rows[sel], r]

    # ---- parameter packing ----
    def g(name):
        return np.asarray(inputs[name], f32)

    params = dict(
        emb_rbf_w=g("emb_rbf_w"), emb_w3=emb_w[2 * EMB:],
        iw_ji=g("iw_ji"), iw_kj=g("iw_kj"), iw_down=g("iw_down"),
        iw_up=g("iw_up"), iw_final=g("iw_final"), iw_res=g("iw_res"),
        iw_rbf1=g("iw_rbf1"), iw_rbf2=g("iw_rbf2"),
        ow_dense=g("ow_dense"),
        ow_out=np.concatenate([g("ow_out"), np.zeros_like(g("ow_out"))], axis=-1),
        ow_rbf_t=np.transpose(g("ow_rbf"), (0, 1, 3, 2)).copy(),  # [3,5,128,6]
    )

    meta = dict(
        cfg=cfg, C=C, APC=APC, NWIN=NWIN, APAD=APAD, EPAD=EPAD,
        NWAVES=NWAVES, B_k=B_k, cumB=cumB, BMAX=BMAX, SLOTS=SLOTS,
        pieces=pieces, NINST=NINST, insts=insts, sched=sched,
        wstart=wstart, nchunk=nchunk,
        win_of_atom=win_of_atom, slot_of_atom=slot_of_atom,
        l_of_old=l_of_old, q_of_old=q_of_old,
    )
    percore = dict(
        e_rbf_fm=np.ascontiguousarray(np.transpose(e_rbf_c, (0, 2, 1))),
        hja_fm=np.ascontiguousarray(np.transpose(hja_c, (0, 2, 1))),
        gidx=gidx_w_c, scat_idx=scat_idx_c, sbfp=sbfp_c,
        s6=S6_c.astype(ml_dtypes.bfloat16),
    )
    return meta, percore, params


# ============================ numpy emulator ============================

def emulate_core(meta, pc, params, q):
    """Replicates the device program for core q in numpy (fp32)."""
    f32 = np.float32
    EPAD, SLOTS, BMAX = meta["EPAD"], meta["SLOTS"], meta["BMAX"]
    NWIN, C = meta["NWIN"], meta["C"]

    def swish(x):
        return (x / (1 + np.exp(-x))).astype(f32)

    e_rbf = pc["e_rbf_fm"][q].T            # [EPAD, 6]
    hja = pc["hja_fm"][q].T                # [EPAD, 128]
    rbf_e = swish(e_rbf @ params["emb_rbf_w"])
    m = swish(hja + rbf_e @ params["emb_w3"])

    gidx = pc["gidx"][q][:16].T.reshape(-1)[:SLOTS].astype(np.int64)
    sbfp = pc["sbfp"][q]
    s6 = pc["s6"][q]

    def out_level(m_, o):
        # Q[f, win, 6*WA] via S6
        Q = np.zeros((NWIN, EMB, 6 * WA), f32)
        for w, lst in enumerate(meta["sched"]):
            for (ii, ch) in lst:
                Q[w] += m_[ch * P:(ch + 1) * P].T @ s6[ii]
        hh = {}
        for k in range(N_KEYS):
            wk = params["ow_rbf_t"][k, o]          # [128, 6]
            acc = np.zeros((EMB, NWIN * WA), f32)
            for r in range(6):
                acc += wk[:, r:r + 1] * Q[:, :, r * WA:(r + 1) * WA] \
                    .transpose(1, 0, 2).reshape(EMB, NWIN * WA)
            x = acc                                  # [128 f, APAD]
            for l in range(3):
                x = swish((params["ow_dense"][k, o, l].T @ x))
            hh[k] = (params["ow_out"][k, o][:, 0] @ x)   # [APAD]
        return hh

    atomwise = [np.zeros(meta["APAD"], f32) for _ in range(N_KEYS)]
    lvl = out_level(m, 0)
    for k in range(N_KEYS):
        atomwise[k] += lvl[k]

    # exchange emulation needs all cores' products; emulated at caller level
    return m, atomwise


def emulate_all(meta, pc, params):
    """Full 8-core emulation incl. exchange; returns [3, C, APAD] atomwise."""
    f32 = np.float32
    C, EPAD, SLOTS, BMAX = meta["C"], meta["EPAD"], meta["SLOTS"], meta["BMAX"]
    NWIN = meta["NWIN"]

    def swish(x):
        return (x / (1 + np.exp(-x))).astype(f32)

    e_rbf = [pc["e_rbf_fm"][q].T for q in range(C)]
    m = []
    for q in range(C):
        rbf_e = swish(e_rbf[q] @ params["emb_rbf_w"])
        m.append(swish(pc["hja_fm"][q].T + rbf_e @ params["emb_w3"]))

    gidx = [pc["gidx"][q][:16].T.reshape(-1)[:SLOTS].astype(np.int64)
            for q in range(C)]
    sidx = [pc["scat_idx"][q][:16].T.reshape(-1).astype(np.int64)
            for q in range(C)]

    def out_level(q, m_, o, atomwise):
        Q = np.zeros((NWIN, EMB, 6 * WA), f32)
        for w, lst in enumerate(meta["sched"]):
            for (ii, ch) in lst:
                Q[w] += m_[ch * P:(ch + 1) * P].T @ pc["s6"][q][ii]
        for k in range(N_KEYS):
            wk = params["ow_rbf_t"][k, o]
            acc = np.zeros((EMB, NWIN * WA), f32)
            for r in range(6):
                acc += wk[:, r:r + 1] * np.transpose(
                    Q[:, :, r * WA:(r + 1) * WA], (1, 0, 2)).reshape(EMB, -1)
            x = acc
            for l in range(3):
                x = swish(params["ow_dense"][k, o, l].T @ x)
            atomwise[k][q] += params["ow_out"][k, o][:, 0] @ x

    atomwise = [np.zeros((C, meta["APAD"]), f32) for _ in range(N_KEYS)]
    for q in range(C):
        out_level(q, m[q], 0, atomwise)

    for c in range(N_CONV):
        x_ji, x_kj_e, prod = [], [], []
        for q in range(C):
            rbf_p = (e_rbf[q] @ params["iw_rbf1"][c]) @ params["iw_rbf2"][c]
            x_ji.append(swish(m[q] @ params["iw_ji"][c]))
            xkj = swish(m[q] @ params["iw_kj"][c])
            xke = swish((xkj * rbf_p) @ params["iw_down"][c])   # [EPAD, 64]
            x_kj_e.append(xke)
            g = xke[gidx[q]].astype(ml_dtypes.bfloat16)
            pr = (g * pc["sbfp"][q][c]).astype(np.float32)
            prod.append(pr)
        # alltoall: recv[q][s*BMAX + j] = prod[s][q*BMAX + j]
        agg = []
        for q in range(C):
            recv = np.concatenate(
                [prod[s][q * BMAX:(q + 1) * BMAX] for s in range(C)])
            ag = np.zeros((EPAD + P, INT_DIM), f32)
            col = 0
            for (k, b0, bsz) in meta["pieces"]:
                rows = np.concatenate(
                    [recv[s * BMAX + meta["cumB"][k] + b0:
                          s * BMAX + meta["cumB"][k] + b0 + bsz]
                     for s in range(C)])
                tg = sidx[q][col:col + C * bsz]
                np.add.at(ag, tg, rows)
                col += C * bsz
            agg.append(ag[:EPAD])
        for q in range(C):
            hh = x_ji[q] + swish(agg[q] @ params["iw_up"][c])
            z = swish(swish(hh @ params["iw_res"][c, 0, 0])
                      @ params["iw_res"][c, 0, 1])
            hh = hh + z
            m[q] = swish(hh @ params["iw_final"][c]) + m[q]
            for r in (1, 2):
                z = swish(swish(m[q] @ params["iw_res"][c, r, 0])
                          @ params["iw_res"][c, r, 1])
                m[q] = m[q] + z
            out_level(q, m[q], c + 1, atomwise)
    return atomwise


def host_finalize(meta, atomwise_list, cfg):
    """atomwise_list: [C] of [3, APAD] -> [NG, 2] adiabatic energies."""
    NA, NG, APM = cfg["NA"], cfg["NG"], cfg["APM"]
    C, APC = meta["C"], meta["APC"]
    win_of_atom, slot_of_atom = meta["win_of_atom"], meta["slot_of_atom"]
    E = np.zeros((N_KEYS, NG), np.float64)
    a = np.arange(NA)
    padpos = win_of_atom * WA + slot_of_atom
    mol = a // APM
    for k in range(N_KEYS):
        vals = np.stack([atomwise_list[q][k] for q in range(C)])  # [C, APAD]
        atom_e = vals[a // APC, padpos]
        np.add.at(E[k], mol, atom_e.astype(np.float64))
    d0, d1, lam = E[0], E[1], E[2]
    tr = 0.5 * (d0 + d1)
    rad = np.sqrt((0.5 * (d0 - d1)) ** 2 + lam * lam)
    out = np.stack([tr - rad, tr + rad], axis=-1).astype(np.float32)
    return out


# ============================ Bass program ============================

def build_program(meta):
    import concourse.bacc as bacc
    import concourse.bass as bass
    import concourse.mybir as mybir
    import concourse.tile as tile
    from concourse.masks import make_identity

    f32 = mybir.dt.float32
    fr = mybir.dt.float32r
    bf16 = mybir.dt.bfloat16
    i16 = mybir.dt.int16
    SILU = mybir.ActivationFunctionType.Silu
    EPAD, SLOTS, BMAX = meta["EPAD"], meta["SLOTS"], meta["BMAX"]
    NWIN, APAD, NINST = meta["NWIN"], meta["APAD"], meta["NINST"]
    C = meta["C"]
    cumB = meta["cumB"]
    pieces = meta["pieces"]
    sched = meta["sched"]
    NMT = EPAD // TILE
    XW = 2 * INT_DIM            # padded bf16 row width of x_kj_e
    scols = sum(C * bsz // 16 for (_, _, bsz) in pieces)

    nc = bacc.Bacc("TRN2", target_bir_lowering=False, debug=False,
                   num_devices=C)

    # ---- I/O (f32r tensors carry plain fp32 bits) ----
    e_rbf_t = nc.dram_tensor("e_rbf_fm", [N_RBF, EPAD], fr, kind="ExternalInput")
    hja_t = nc.dram_tensor("hja_fm", [EMB, EPAD], fr, kind="ExternalInput")
    gidx_t = nc.dram_tensor("gidx", [P, SLOTS // 16], i16, kind="ExternalInput")
    sidx_t = nc.dram_tensor("scat_idx", [P, scols], i16, kind="ExternalInput")
    sbfp_t = nc.dram_tensor("sbfp", [N_CONV, SLOTS, INT_DIM], bf16,
                            kind="ExternalInput")
    s6_t = nc.dram_tensor("s6", [NINST, P, 6 * WA], bf16, kind="ExternalInput")
    p_emb_rbf = nc.dram_tensor("emb_rbf_w", [N_RBF, EMB], fr, kind="ExternalInput")
    p_emb_w3 = nc.dram_tensor("emb_w3", [EMB, EMB], fr, kind="ExternalInput")
    p_ji = nc.dram_tensor("iw_ji", [N_CONV, EMB, EMB], fr, kind="ExternalInput")
    p_kj = nc.dram_tensor("iw_kj", [N_CONV, EMB, EMB], fr, kind="ExternalInput")
    p_down = nc.dram_tensor("iw_down", [N_CONV, EMB, INT_DIM], fr,
                            kind="ExternalInput")
    p_up = nc.dram_tensor("iw_up", [N_CONV, INT_DIM, EMB], fr,
                          kind="ExternalInput")
    p_final = nc.dram_tensor("iw_final", [N_CONV, EMB, EMB], fr,
                             kind="ExternalInput")
    p_res = nc.dram_tensor("iw_res", [N_CONV, 3, 2, EMB, EMB], fr,
                           kind="ExternalInput")
    p_rbf1 = nc.dram_tensor("iw_rbf1", [N_CONV, N_RBF, BEMB], fr,
                            kind="ExternalInput")
    p_rbf2 = nc.dram_tensor("iw_rbf2", [N_CONV, BEMB, EMB], fr,
                            kind="ExternalInput")
    p_owd = nc.dram_tensor("ow_dense", [N_KEYS, NLEVEL, 3, EMB, EMB], fr,
                           kind="ExternalInput")
    p_owo = nc.dram_tensor("ow_out", [N_KEYS, NLEVEL, EMB, 2], fr,
                           kind="ExternalInput")
    p_owr = nc.dram_tensor("ow_rbf_t", [N_KEYS, NLEVEL, EMB, N_RBF], f32,
                           kind="ExternalInput")
    atw_t = nc.dram_tensor("atomwise", [N_KEYS, APAD], f32,
                           kind="ExternalOutput")

    with tile.TileContext(nc) as tc:
        with (
            tc.tile_pool(name="wk", bufs=3) as wk,
            tc.tile_pool(name="hhp", bufs=1) as hhp,
            tc.tile_pool(name="cst", bufs=1) as cst,
            tc.tile_pool(name="par", bufs=2) as par,
            tc.tile_pool(name="gth", bufs=2) as gth,
            tc.tile_pool(name="scp", bufs=2) as scp,
            tc.tile_pool(name="idxp", bufs=2) as idxp,
            tc.tile_pool(name="s6p", bufs=2) as s6p,
            tc.tile_pool(name="mtp", bufs=2) as mtp,
            tc.tile_pool(name="ps", bufs=2, space="PSUM") as ps,
            tc.tile_pool(name="ps_s", bufs=2, space="PSUM") as ps_s,
            tc.tile_pool(name="ps_t", bufs=2, space="PSUM") as ps_t,
            tc.tile_pool(name="dram", bufs=1, space="DRAM") as dram,
        ):
            identf = cst.tile([P, P], f32, tag="identf")
            make_identity(nc, identf[:])
            ident = cst.tile([P, P], fr, tag="ident")
            nc.vector.tensor_copy(ident[:], identf[:])
            m_sb = cst.tile([EMB, EPAD], fr, tag="m")
            Q_sb = cst.tile([EMB, NWIN * 6 * WA], f32, tag="Q")
            atw_sb = cst.tile([P, APAD + (APAD & 1)], f32, tag="atw")
            nc.vector.memset(atw_sb[:], 0.0)
            zer_sb = cst.tile([P, 2 * INT_DIM], f32, tag="zer")
            nc.vector.memset(zer_sb[:], 0.0)

            xkje_d = dram.tile([EPAD, XW], bf16, tag="xkje")
            zero_d = dram.tile([EPAD + P, INT_DIM], f32, tag="zerod")
            xji_d = dram.tile([EMB, EPAD], bf16, tag="xji")
            agg_d = dram.tile([EPAD + P, INT_DIM], f32, tag="agg")
            a2a_in = dram.tile([SLOTS, INT_DIM], bf16, tag="a2ai")
            a2a_out = dram.tile([C * BMAX, INT_DIM], bf16, tag="a2ao")

            def act(dst, src, func=SILU):
                nc.scalar.activation(dst, src, func)

            # fill zero template once
            zv = zero_d[:].rearrange("(b p) e -> p b e", p=P)
            nzb0 = (EPAD + P) // P
            for b0 in range(0, nzb0, 2):
                bn = min(2, nzb0 - b0)
                nc.sync.dma_start(
                    zv[:, b0:b0 + bn, :],
                    zer_sb[:].rearrange("p (b e) -> p b e", e=INT_DIM)[:, :bn, :])

            # -------- embedding --------
            embw_sb = par.tile([N_RBF, EMB], fr, tag="p0")
            nc.sync.dma_start(embw_sb[:], p_emb_rbf[:])
            w3_sb = par.tile([EMB, EMB], fr, tag="p1")
            nc.sync.dma_start(w3_sb[:], p_emb_w3[:])
            for t in range(0 if os.environ.get("K_SKIP_EMB", "0") == "1" else NMT):
                sl = slice(t * TILE, (t + 1) * TILE)
                erbf = wk.tile([N_RBF, TILE], fr, tag="w")
                nc.sync.dma_start(erbf[:], e_rbf_t[:, sl])
                psr = ps.tile([P, TILE], f32, space="PSUM", tag="big")
                for h in range(2):
                    hsl = slice(h * 512, (h + 1) * 512)
                    nc.tensor.matmul(psr[:, hsl], lhsT=embw_sb[:],
                                     rhs=erbf[:, hsl], start=True, stop=True)
                rbfe = wk.tile([P, TILE], fr, tag="w")
                act(rbfe[:], psr[:])
                psm = ps.tile([P, TILE], f32, space="PSUM", tag="big")
                hj = wk.tile([P, TILE], fr, tag="w")
                nc.sync.dma_start(hj[:], hja_t[:, sl])
                for h in range(2):
                    hsl = slice(h * 512, (h + 1) * 512)
                    nc.tensor.matmul(psm[:, hsl], lhsT=w3_sb[:],
                                     rhs=rbfe[:, hsl], start=True, stop=False)
                    nc.tensor.matmul(psm[:, hsl], lhsT=ident[:],
                                     rhs=hj[:, hsl], start=False, stop=True)
                act(m_sb[:, sl], psm[:])

            # -------- out-block --------
            def out_block(lvl):
                for w in range(NWIN):
                    psq = ps_s.tile([P, 6 * WA], f32, space="PSUM", tag="small")
                    lst = sched[w]
                    # instances of a window are consecutive in s6_t; load in
                    # blocks of 4
                    blocks = [lst[i:i + 2] for i in range(0, len(lst), 2)]
                    n = 0
                    for blk in blocks:
                        i0 = blk[0][0]
                        s6i = s6p.tile([P, 2, 6 * WA], bf16, tag="s6")
                        nc.sync.dma_start(s6i[:, :len(blk), :],
                                          s6_t[i0:i0 + len(blk)]
                                          .rearrange("i p x -> p i x"))
                        for j, (ii, ch) in enumerate(blk):
                            pst = ps_t.tile([P, P], fr, space="PSUM", tag="t")
                            nc.tensor.transpose(
                                pst[:], m_sb[:, ch * P:(ch + 1) * P], ident[:])
                            mt = mtp.tile([P, P], bf16, tag="mt")
                            nc.vector.tensor_copy(mt[:], pst[:])
                            nc.tensor.matmul(psq[:], lhsT=mt[:],
                                             rhs=s6i[:, j, :],
                                             start=(n == 0),
                                             stop=(n == len(lst) - 1))
                            n += 1
                    nc.vector.tensor_copy(
                        Q_sb[:, w * 6 * WA:(w + 1) * 6 * WA], psq[:])
                APE = APAD + (APAD & 1)
                chunks = [(o, min(512, APE - o)) for o in range(0, APE, 512)]
                for k in range(N_KEYS):
                    wko = par.tile([EMB, N_RBF], f32, tag="p2")
                    nc.sync.dma_start(wko[:], p_owr[k, lvl])
                    hh = hhp.tile([EMB, APE], fr, tag="hh")
                    if APE != APAD:
                        nc.vector.tensor_copy(hh[:, APAD:APE],
                                              zer_sb[:, :APE - APAD])
                    qv = Q_sb[:].rearrange("p (w rs) -> p w rs", rs=6 * WA)
                    hv = hh[:, :APAD].rearrange("p (w a) -> p w a", a=WA)
                    for r in range(6):
                        qr = qv[:, :, r * WA:(r + 1) * WA]
                        if r == 0:
                            nc.vector.tensor_scalar_mul(hv, qr, wko[:, 0:1])
                        else:
                            nc.vector.scalar_tensor_tensor(
                                hv, qr, wko[:, r:r + 1], hv,
                                op0=mybir.AluOpType.mult,
                                op1=mybir.AluOpType.add)
                    for l in range(3):
                        wd = par.tile([EMB, EMB], fr, tag="p3")
                        nc.sync.dma_start(wd[:], p_owd[k, lvl, l])
                        for (o, w_) in chunks:
                            psd = ps_s.tile([P, 512], f32, space="PSUM",
                                            tag="small")
                            nc.tensor.matmul(psd[:, :w_], lhsT=wd[:],
                                             rhs=hh[:, o:o + w_],
                                             start=True, stop=True)
                            act(hh[:, o:o + w_], psd[:, :w_])
                    wo = par.tile([EMB, 2], fr, tag="p2b")
                    nc.sync.dma_start(wo[:], p_owo[k, lvl])
                    for (o, w_) in chunks:
                        pse = ps_s.tile([2, 512], f32, space="PSUM", tag="small")
                        nc.tensor.matmul(pse[:, :w_], lhsT=wo[:],
                                         rhs=hh[:, o:o + w_],
                                         start=True, stop=True)
                        av = atw_sb[32 * k:32 * k + 1, o:o + w_]
                        nc.vector.tensor_add(av, av, pse[0:1, :w_])

            skip_out = os.environ.get("K_SKIP_OUT", "0") == "1"
            skip_ang = os.environ.get("K_SKIP_ANG", "0") == "1"
            skip_stgA = os.environ.get("K_SKIP_STGA", "0") == "1"
            skip_stgD = os.environ.get("K_SKIP_STGD", "0") == "1"
            # -------- conv loop --------
            for c in range(N_CONV):
                wji = par.tile([EMB, EMB], fr, tag="c0")
                nc.sync.dma_start(wji[:], p_ji[c])
                wkj = par.tile([EMB, EMB], fr, tag="c1")
                nc.sync.dma_start(wkj[:], p_kj[c])
                wdn = par.tile([EMB, INT_DIM], fr, tag="c2")
                nc.sync.dma_start(wdn[:], p_down[c])
                wr1 = par.tile([N_RBF, BEMB], fr, tag="c3")
                nc.sync.dma_start(wr1[:], p_rbf1[c])
                wr2 = par.tile([BEMB, EMB], fr, tag="c4")
                nc.sync.dma_start(wr2[:], p_rbf2[c])

                # stage A: x_ji spill, x_kj_e (bf16-padded rows) to HBM
                for t in range(0 if skip_stgA else NMT):
                    sl = slice(t * TILE, (t + 1) * TILE)
                    psj = ps.tile([P, TILE], f32, space="PSUM", tag="big")
                    for h in range(2):
                        hsl = slice(h * 512, (h + 1) * 512)
                        nc.tensor.matmul(
                            psj[:, hsl], lhsT=wji[:],
                            rhs=m_sb[:, t * TILE + h * 512:
                                     t * TILE + (h + 1) * 512],
                            start=True, stop=True)
                    xji = wk.tile([P, TILE], bf16, tag="w")
                    act(xji[:], psj[:])
                    nc.sync.dma_start(xji_d[:, sl], xji[:])

                    psk = ps.tile([P, TILE], f32, space="PSUM", tag="big")
                    for h in range(2):
                        hsl = slice(h * 512, (h + 1) * 512)
                        nc.tensor.matmul(
                            psk[:, hsl], lhsT=wkj[:],
                            rhs=m_sb[:, t * TILE + h * 512:
                                     t * TILE + (h + 1) * 512],
                            start=True, stop=True)
                    xkj = wk.tile([P, TILE], fr, tag="w")
                    act(xkj[:], psk[:])

                    t1 = wk.tile([BEMB, TILE], fr, tag="w")
                    erbf = wk.tile([N_RBF, TILE], fr, tag="w")
                    nc.sync.dma_start(erbf[:], e_rbf_t[:, sl])
                    for h in range(2):
                        hsl = slice(h * 512, (h + 1) * 512)
                        ps1 = ps_s.tile([BEMB, 512], f32, space="PSUM",
                                        tag="small")
                        nc.tensor.matmul(ps1[:], lhsT=wr1[:], rhs=erbf[:, hsl],
                                         start=True, stop=True)
                        nc.vector.tensor_copy(t1[:, hsl], ps1[:])
                    psp = ps.tile([P, TILE], f32, space="PSUM", tag="big")
                    for h in range(2):
                        hsl = slice(h * 512, (h + 1) * 512)
                        nc.tensor.matmul(psp[:, hsl], lhsT=wr2[:],
                                         rhs=t1[:, hsl], start=True, stop=True)
                    xm = wk.tile([P, TILE], fr, tag="w")
                    nc.vector.tensor_mul(xm[:], xkj[:], psp[:])

                    pse4 = ps_s.tile([P, 512], f32, space="PSUM", tag="small")
                    for i in range(8):
                        nc.tensor.matmul(pse4[:, i * 64:(i + 1) * 64],
                                         lhsT=xm[:, i * P:(i + 1) * P],
                                         rhs=wdn[:], start=True, stop=True)
                    xke = wk.tile([P, 512], bf16, tag="w")
                    act(xke[:], pse4[:])
                    nc.sync.dma_start(
                        xkje_d[:].rearrange("(b p) x -> p b x", p=P)
                        [:, t * 8:(t + 1) * 8, :INT_DIM],
                        xke[:].rearrange("p (b e) -> p b e", e=INT_DIM))

                # stage B: gather + products (bf16)
                for g in range(0 if skip_ang else SLOTS // GCH):
                    gi = idxp.tile([P, GCH // 16], i16, tag="gi")
                    nc.sync.dma_start(
                        gi[:], gidx_t[:, g * GCH // 16:(g + 1) * GCH // 16])
                    gb = gth.tile([P, GCH // P, XW], bf16, tag="gb")
                    nc.gpsimd.dma_gather(
                        gb[:], xkje_d[:], gi[:], num_idxs=GCH,
                        num_idxs_reg=GCH, elem_size=XW,
                        single_packet=False)
                    sp = gth.tile([P, GCH // P, INT_DIM], bf16, tag="sp")
                    nc.sync.dma_start(
                        sp[:],
                        sbfp_t[c, g * GCH:(g + 1) * GCH, :]
                        .rearrange("(b p) e -> p b e", p=P))
                    nc.vector.tensor_mul(sp[:], gb[:, :, :INT_DIM], sp[:])
                    nc.sync.dma_start(
                        a2a_in[g * GCH:(g + 1) * GCH, :]
                        .rearrange("(b p) e -> p b e", p=P), sp[:])

                if not skip_ang:
                    nc.gpsimd.collective_compute(
                        "AllToAll", mybir.AluOpType.bypass,
                        replica_groups=[list(range(C))],
                        ins=[a2a_in[:C * BMAX, :].opt()],
                        outs=[a2a_out[:C * BMAX, :].opt()])

                # out-block for level c overlaps the collective + scatter
                if not skip_out:
                    out_block(c)

                # zero agg from template (single big DRAM->DRAM copy)
                if not skip_ang:
                    nc.sync.dma_start(agg_d[:], zero_d[:])

                # stage C: scatter waves
                col = 0
                for (k, b0, bsz) in ([] if skip_ang else pieces):
                    rows = C * bsz
                    si = idxp.tile([P, rows // 16], i16, tag="si")
                    nc.sync.dma_start(si[:], sidx_t[:, col:col + rows // 16])
                    sc = scp.tile([P, rows // P, INT_DIM], f32, tag="sc")
                    nb = bsz // P
                    for s_src in range(C):
                        r0 = s_src * BMAX + int(cumB[k]) + b0
                        nc.gpsimd.dma_start(
                            sc[:, s_src * nb:(s_src + 1) * nb, :],
                            a2a_out[r0:r0 + bsz, :]
                            .rearrange("(b p) e -> p b e", p=P))
                    nc.gpsimd.dma_scatter_add(
                        agg_d[:], sc[:], si[:], num_idxs=rows,
                        num_idxs_reg=rows, elem_size=INT_DIM,
                        single_packet=False)
                    col += rows // 16

                # stage D: aggregate back + rest of conv
                wup = par.tile([INT_DIM, EMB], fr, tag="c0")
                nc.sync.dma_start(wup[:], p_up[c])
                wfi = par.tile([EMB, EMB], fr, tag="c1")
                nc.sync.dma_start(wfi[:], p_final[c])
                wres = []
                for r in range(3):
                    for s_ in range(2):
                        wres_t = par.tile([EMB, EMB], fr, tag=f"r{r}{s_}")
                        wres.append(wres_t)
                for r in range(3):
                    for s_ in range(2):
                        nc.sync.dma_start(wres[r * 2 + s_][:], p_res[c, r, s_])

                for t in range(0 if skip_stgD else NMT):
                    sl = slice(t * TILE, (t + 1) * TILE)
                    afm = wk.tile([INT_DIM, TILE], fr, tag="w")
                    arb = mtp.tile([P, 8, INT_DIM], f32, tag="ar")
                    nc.sync.dma_start(
                        arb[:],
                        agg_d[t * TILE:(t + 1) * TILE, :]
                        .rearrange("(b p) e -> p b e", p=P))
                    for i in range(8):
                        pst = ps_t.tile([INT_DIM, P], f32, space="PSUM", tag="t")
                        nc.tensor.transpose(pst[:], arb[:, i, :], identf[:])
                        nc.vector.tensor_copy(afm[:, i * P:(i + 1) * P], pst[:])
                    psu = ps.tile([P, TILE], f32, space="PSUM", tag="big")
                    for h in range(2):
                        hsl = slice(h * 512, (h + 1) * 512)
                        nc.tensor.matmul(psu[:, hsl], lhsT=wup[:],
                                         rhs=afm[:, hsl], start=True, stop=True)
                    hh = wk.tile([P, TILE], fr, tag="w")
                    act(hh[:], psu[:])
                    xji = wk.tile([P, TILE], bf16, tag="w")
                    nc.sync.dma_start(xji[:], xji_d[:, sl])
                    nc.vector.tensor_add(hh[:], hh[:], xji[:])

                    def res_pair(vin, w0, w1):
                        psa = ps.tile([P, TILE], f32, space="PSUM", tag="big")
                        for h in range(2):
                            hsl = slice(h * 512, (h + 1) * 512)
                            nc.tensor.matmul(psa[:, hsl], lhsT=w0[:],
                                             rhs=vin[:, hsl], start=True,
                                             stop=True)
                        zz = wk.tile([P, TILE], fr, tag="w")
                        act(zz[:], psa[:])
                        psb = ps.tile([P, TILE], f32, space="PSUM", tag="big")
                        for h in range(2):
                            hsl = slice(h * 512, (h + 1) * 512)
                            nc.tensor.matmul(psb[:, hsl], lhsT=w1[:],
                                             rhs=zz[:, hsl], start=True,
                                             stop=True)
                        act(zz[:], psb[:])
                        nc.vector.tensor_add(vin[:], vin[:], zz[:])

                    res_pair(hh, wres[0], wres[1])
                    psf = ps.tile([P, TILE], f32, space="PSUM", tag="big")
                    for h in range(2):
                        hsl = slice(h * 512, (h + 1) * 512)
                        nc.tensor.matmul(psf[:, hsl], lhsT=wfi[:],
                                         rhs=hh[:, hsl], start=True, stop=True)
                    fz = wk.tile([P, TILE], fr, tag="w")
                    act(fz[:], psf[:])
                    mview = m_sb[:, sl]
                    nc.vector.tensor_add(mview, mview, fz[:])
                    res_pair(mview, wres[2], wres[3])
                    res_pair(mview, wres[4], wres[5])

            if not skip_out:
                out_block(N_CONV)

            for k in range(N_KEYS):
                nc.sync.dma_start(atw_t[k:k + 1, :],
                                  atw_sb[32 * k:32 * k + 1, :APAD])

    nc.compile()
    return nc


# ============================ runner ============================

_CACHE = {}
_LAST_RESULT = None


def _run(meta, percore, params, cfg):
    from concourse.bass_utils import run_bass_kernel_spmd
    key = (meta["EPAD"], meta["SLOTS"], meta["BMAX"], meta["NINST"],
           tuple(meta["B_k"]), len(meta["pieces"]))
    if key not in _CACHE:
        _CACHE[key] = build_program(meta)
    nc = _CACHE[key]
    C = meta["C"]
    in_maps = []
    for q in range(C):
        im = dict(
            e_rbf_fm=percore["e_rbf_fm"][q], hja_fm=percore["hja_fm"][q],
            gidx=percore["gidx"][q], scat_idx=percore["scat_idx"][q],
            sbfp=percore["sbfp"][q], s6=percore["s6"][q],
        )
        im.update({k: v for k, v in params.items()})
        im["emb_rbf_w"] = params["emb_rbf_w"]
        in_maps.append(im)
    trace = os.environ.get("KERNEL_TRACE", "0") == "1"
    res = run_bass_kernel_spmd(nc, in_maps, core_ids=list(range(C)),
                               trace=trace)
    global _LAST_RESULT
    _LAST_RESULT = res
    return [r["atomwise"] for r in res.results]


def kernel(**inputs):
    cfg = FULL_CFG
    meta, percore, params = host_prep(inputs, cfg)
    atw = _run(meta, percore, params, cfg)
    return host_finalize(meta, atw, cfg)


def _run_timed(meta, percore, params, iters=6):
    """Steady-state wall-clock timing of the jitted 8-core executable with
    device-resident inputs. Returns (results, [per-iter seconds])."""
    import time
    import jax
    import numpy as np_
    from concourse import bass2jax, mybir
    from jax.sharding import Mesh, PartitionSpec
    from jax.experimental.shard_map import shard_map

    key = (meta["EPAD"], meta["SLOTS"], meta["BMAX"], meta["NINST"],
           tuple(meta["B_k"]), len(meta["pieces"]))
    if key not in _CACHE:
        _CACHE[key] = build_program(meta)
    nc = _CACHE[key]
    C = meta["C"]
    bass2jax.install_neuronx_cc_hook()

    in_names, out_names, out_avals, zero_outs = [], [], [], []
    partition_name = (nc.partition_id_tensor.name
                      if nc.partition_id_tensor else None)
    for alloc in nc.m.functions[0].allocations:
        if not isinstance(alloc, mybir.MemoryLocationSet):
            continue
        name = alloc.memorylocations[0].name
        if alloc.kind == "ExternalInput":
            if name != partition_name:
                in_names.append(name)
        elif alloc.kind == "ExternalOutput":
            shape = tuple(alloc.tensor_shape)
            dtype = mybir.dt.np(alloc.dtype)
            out_names.append(name)
            out_avals.append(jax.core.ShapedArray(shape, dtype))
            zero_outs.append(np_.zeros(shape, dtype))
    n_params = len(in_names)
    all_in = in_names + out_names
    if partition_name is not None:
        all_in.append(partition_name)

    def _body(*args):
        operands = list(args)
        if partition_name is not None:
            operands.append(bass2jax.partition_id_tensor())
        return tuple(bass2jax._bass_exec_p.bind(
            *operands, out_avals=tuple(out_avals), in_names=tuple(all_in),
            out_names=tuple(out_names), lowering_input_output_aliases=(),
            sim_require_finite=True, sim_require_nnan=True, nc=nc))

    devices = jax.devices()[:C]
    mesh = Mesh(np_.asarray(devices), ("core",))
    n_outs = len(out_names)
    sharded = jax.jit(
        shard_map(_body, mesh=mesh,
                  in_specs=(PartitionSpec("core"),) * (n_params + n_outs),
                  out_specs=(PartitionSpec("core"),) * n_outs,
                  check_rep=False),
        keep_unused=True)

    def in_map(q):
        im = dict(
            e_rbf_fm=percore["e_rbf_fm"][q], hja_fm=percore["hja_fm"][q],
            gidx=percore["gidx"][q], scat_idx=percore["scat_idx"][q],
            sbfp=percore["sbfp"][q], s6=percore["s6"][q])
        im.update(params)
        return im

    maps = [in_map(q) for q in range(C)]
    from jax.sharding import NamedSharding
    concat_in = []
    for i, name in enumerate(in_names):
        arr = np_.concatenate([np_.asarray(maps[c][name]) for c in range(C)],
                              axis=0)
        concat_in.append(jax.device_put(
            arr, NamedSharding(mesh, PartitionSpec("core"))))
    concat_zeros = [
        jax.device_put(np_.zeros((C * z.shape[0], *z.shape[1:]), z.dtype),
                       NamedSharding(mesh, PartitionSpec("core")))
        for z in zero_outs]

    times = []
    outs = None
    for it in range(iters):
        t0 = time.perf_counter()
        outs = sharded(*concat_in, *concat_zeros)
        jax.block_until_ready(outs)
        times.append(time.perf_counter() - t0)
    results = [
        {name: np_.asarray(outs[i]).reshape(C, *out_avals[i].shape)[c]
         for i, name in enumerate(out_names)} for c in range(C)]
    return [r["atomwise"] for r in results], times



# revision 3
# speedup vs baseline: 1.8525x; 1.8525x over previous
"""DimeNet-diabat Trainium2 kernel: 8-core SPMD Bass implementation.

Sharding: edges/angles/atoms partitioned by owner atom core (atom a -> core
a // (NA/8)); molecules never straddle cores. Parameters replicated.

Device pipeline (per core, identical SPMD program):
  - Edge MLP chain feature-major ([128 feat partitions, edges free]).
  - Angle message passing: dma_gather of local x_kj_e rows in (dst core,
    dst 256-edge window, seat) slot order, AllToAll of the raw gathered
    rows, then receiver-side multiply by sbf_p and PE indicator-matmul
    segment aggregation (one [128,64]x[128,256] matmul per (window, src)
    segment accumulating into PSUM) -- no DMA scatter at all.
  - Out-blocks: per-atom segment sums as PE matmuls (transposed-m chunks
    against a static e_rbf-scaled indicator "S6"), atoms rebalanced into
    edge-count-balanced 21-atom windows so the schedule is static across
    cores; dense heads on local atoms.
Host: index relabeling (with per-256-window angle-count balancing so every
(src, window) segment fits the shared 128-seat budget), basis functions,
embedding gather, molecule sums, final 2x2 eigendecomposition.
"""

import os
import ml_dtypes
import numpy as np

# ---------------- problem constants (hardcoded from spec) ----------------
CUTOFF = 5.0
ENV_P = 6
N_RBF, N_SPHER, L_SPHER = 6, 6, 7
SBF = N_SPHER * L_SPHER
EMB, INT_DIM, BEMB = 128, 64, 8
N_CONV, N_KEYS = 4, 3
NLEVEL = N_CONV + 1

FULL_CFG = dict(NA=8000, NE=200000, NW=600000, NG=80, APM=100)

NCORES = 8
P = 128
WA = 43                 # atoms per window (6*43=258 free dim, f32r-fast)
TILE = 1024             # edge macro-tile (2 PSUM banks per activation span)
GCH = 4096              # gather chunk (slots)
SWE = 256               # edges per aggregation window
BSEG = 128              # seat budget per (src, window) segment


# ============================ host preprocessing ============================

def _envelope(x):
    p = ENV_P
    a = -(p + 1) * (p + 2) / 2.0
    b = float(p * (p + 2))
    c = -p * (p + 1) / 2.0
    with np.errstate(divide="ignore"):
        env = 1.0 / x + a * x ** (p - 1) + b * x ** p + c * x ** (p + 1)
    return np.where(x < 1.0, env, 0.0).astype(np.float32)


def _wrap16(idx):
    """int16 index list -> [128, ceil(n/16)] wrapped (w -> [w%16, w//16]),
    replicated across the 8 Q7 cores."""
    n = len(idx)
    cols = -(-n // 16)
    flat = np.zeros(cols * 16, np.int16)
    flat[:n] = np.asarray(idx, np.int16)
    buf = flat.reshape(cols, 16).T.copy()
    return np.tile(buf, (8, 1)).copy()


def _roundup(x, m):
    return int(-(-x // m) * m)


def host_prep(inputs, cfg):
    NA, NE, NW = cfg["NA"], cfg["NE"], cfg["NW"]
    NG, APM = cfg["NG"], cfg["APM"]
    C = NCORES
    APC = NA // C
    NWIN = -(-APC // WA)
    assert NA % C == 0 and APC % APM == 0

    f32 = np.float32
    xyz = np.asarray(inputs["xyz"], f32)
    nbr = np.asarray(inputs["nbr_list"], np.int64)
    ang_l = np.asarray(inputs["angle_list"], np.int64)
    kj_idx = np.asarray(inputs["kj_idx"], np.int64)
    ji_idx = np.asarray(inputs["ji_idx"], np.int64)
    z = np.asarray(inputs["z"], np.int64)

    # ---- geometry / basis ----
    d = np.linalg.norm(xyz[nbr[:, 0]] - xyz[nbr[:, 1]], axis=-1).astype(f32)
    xs = d / f32(CUTOFF)
    n_ar = np.arange(1, N_RBF + 1, dtype=f32)
    e_rbf = (_envelope(xs)[:, None]
             * np.sin(np.pi * n_ar[None, :] * xs[:, None])).astype(f32)

    r_ji = xyz[ang_l[:, 0]] - xyz[ang_l[:, 1]]
    r_jk = xyz[ang_l[:, 2]] - xyz[ang_l[:, 1]]
    cos_t = np.sum(r_ji * r_jk, axis=-1)
    cr = np.cross(r_ji, r_jk)
    sin_t = np.sqrt(np.sum(cr * cr, axis=-1) + 1e-12)
    alpha = np.arctan2(sin_t, cos_t).astype(f32)
    x_kj = xs[kj_idx]
    ns = np.arange(1, N_SPHER + 1, dtype=f32)
    rad = _envelope(x_kj)[:, None] * np.sin(np.pi * ns[None, :] * x_kj[:, None])
    ls = np.arange(L_SPHER, dtype=f32)
    ang_b = np.cos(ls[None, :] * alpha[:, None])
    a_sbf = (ang_b[:, :, None] * rad[:, None, :]).reshape(NW, SBF).astype(f32)

    # ---- embedding gather (host) ----
    emb_z = np.asarray(inputs["emb_z"], f32)
    emb_w = np.asarray(inputs["emb_w"], f32)
    h = emb_z[z]
    hja = (h[nbr[:, 1]] @ emb_w[:EMB]
           + h[nbr[:, 0]] @ emb_w[EMB:2 * EMB]).astype(f32)

    # ---- atom window balancing (per core) ----
    i_atom = nbr[:, 0]
    deg = np.bincount(i_atom, minlength=NA)
    win_of_atom = np.empty(NA, np.int64)
    slot_of_atom = np.empty(NA, np.int64)   # position within window (0..WA-1)
    budgets = np.zeros((C, NWIN), np.int64)
    for q in range(C):
        a0 = q * APC
        order = np.argsort(-deg[a0:a0 + APC], kind="stable")
        fill = np.zeros(NWIN, np.int64)
        cnt = np.zeros(NWIN, np.int64)
        for a in order:
            cand = np.flatnonzero(cnt < WA)
            w = cand[np.argmin(fill[cand])]
            win_of_atom[a0 + a] = w
            slot_of_atom[a0 + a] = cnt[w]
            fill[w] += deg[a0 + a]
            cnt[w] += 1
        budgets[q] = fill
    budget_w = budgets.max(axis=0)          # shared static budgets [NWIN]
    wstart = np.zeros(NWIN + 1, np.int64)
    wstart[1:] = np.cumsum(budget_w)
    EPAD = _roundup(int(wstart[-1]), TILE)
    APAD = NWIN * WA
    NSW = EPAD // SWE
    BMAX = NSW * BSEG
    SLOTS = C * BMAX
    NT = EPAD // TILE
    SWT = TILE // SWE

    # ---- edge position assignment ----
    # Edge owner core + atom window are fixed; the position within the
    # atom-window range is free.  Choose it so per-(src, 256-edge window)
    # angle counts stay under the BSEG seat budget (greedy LPT over the
    # 256-buckets each range overlaps).
    owner = i_atom // APC
    q_of_old = owner
    a_src = q_of_old[kj_idx]
    a_dst = q_of_old[ji_idx]
    ce = np.zeros((NE, C), np.int32)
    np.add.at(ce, (ji_idx, a_src), 1)
    ce_tot = ce.sum(axis=1)

    l_of_old = np.empty(NE, np.int64)
    win_of_edge = win_of_atom[i_atom]
    for q in range(C):
        loads = np.zeros((NSW, C), np.int64)
        for w in range(NWIN):
            sel = np.flatnonzero((owner == q) & (win_of_edge == w))
            p0, p1 = int(wstart[w]), int(wstart[w]) + int(budget_w[w])
            # bucket pieces covered by [p0, p1)
            pieces = []          # [bucket, lo, cap_left]
            lo = p0
            while lo < p1:
                b = lo // SWE
                hi = min((b + 1) * SWE, p1)
                pieces.append([b, lo, hi - lo])
                lo = hi
            order = sel[np.argsort(-ce_tot[sel], kind="stable")]
            assign = [[] for _ in pieces]
            for e in order:
                cvec = ce[e]
                best, bscore = -1, None
                for pi, (b, _, cap) in enumerate(pieces):
                    if cap <= 0:
                        continue
                    score = int((loads[b] + cvec).max())
                    if bscore is None or score < bscore:
                        best, bscore = pi, score
                pb = pieces[best]
                assign[best].append(e)
                pb[2] -= 1
                loads[pb[0]] += cvec
            for pi, (b, lo_, _) in enumerate(pieces):
                es = assign[pi]
                if es:
                    l_of_old[es] = lo_ + np.arange(len(es))

    # per-core local edge arrays (padded layout)
    e_rbf_c = np.zeros((C, EPAD, N_RBF), f32)
    hja_c = np.zeros((C, EPAD, EMB), f32)
    awin_c = np.full((C, EPAD, 2), -1, np.int64)      # (window, slot) per edge
    li_all = l_of_old
    e_rbf_c[q_of_old, li_all] = e_rbf
    hja_c[q_of_old, li_all] = hja
    awin_c[q_of_old, li_all, 0] = win_of_atom[i_atom]
    awin_c[q_of_old, li_all, 1] = slot_of_atom[i_atom]

    # ---- angle slots: (dst, window, seat) per sender ----
    l_ji = l_of_old[ji_idx]
    l_kj = l_of_old[kj_idx]
    sw = l_ji // SWE
    tgt = l_ji % SWE
    gkey = (a_src * C + a_dst) * NSW + sw
    og = np.lexsort((l_kj, gkey))
    kk = gkey[og]
    chg = np.r_[0, np.flatnonzero(np.diff(kk)) + 1]
    glen = np.diff(np.r_[chg, NW])
    assert glen.max() <= BSEG, f"segment overflow: {glen.max()} > {BSEG}"
    pos = np.empty(NW, np.int64)
    pos[og] = np.arange(NW) - np.repeat(chg, glen)
    slot = a_dst * BMAX + sw * BSEG + pos             # sender-local slot

    gidx_c = np.zeros((C, SLOTS), np.int16)
    gidx_c[a_src, slot] = l_kj.astype(np.int16)
    gidx_w_c = np.stack([_wrap16(gidx_c[q]) for q in range(C)])

    # ---- receiver arrays: sbf_p + seat targets in recv-tile order ----
    iw_sbf1 = np.asarray(inputs["iw_sbf1"], f32)
    iw_sbf2 = np.asarray(inputs["iw_sbf2"], f32)
    wc = np.einsum("csb,cbi->csi", iw_sbf1, iw_sbf2)
    # recv flat index (on dst core): t*SWT*C*BSEG + (src*SWT + swl)*BSEG + pos
    t_of = sw // SWT
    swl_of = sw % SWT
    ridx = (t_of * (SWT * C) + a_src * SWT + swl_of) * BSEG + pos
    sbfp_c = np.zeros((C, N_CONV, SLOTS, INT_DIM), ml_dtypes.bfloat16)
    tgt_c = np.full((C, NT, BSEG, C * SWT), 999.0, np.float32)
    for q in range(C):
        m = a_dst == q
        sp = np.einsum("ws,csi->cwi", a_sbf[m], wc)       # [4, n, 64]
        sbfp_c[q][:, ridx[m], :] = sp.astype(ml_dtypes.bfloat16)
        tgt_c[q, t_of[m], pos[m], a_src[m] * SWT + swl_of[m]] = tgt[m]

    # ---- S6 static indicator + schedule (shared structure) ----
    nchunk = EPAD // P
    insts = []                    # (chunk, window)
    sched = [[] for _ in range(NWIN)]
    for ch in range(nchunk):
        lo, hi = ch * P, ch * P + P - 1
        w0 = int(np.searchsorted(wstart[1:], lo, side="right"))
        w1 = int(np.searchsorted(wstart[1:], hi, side="right"))
        w1 = min(w1, NWIN - 1)
        w0 = min(w0, NWIN - 1)
        for w in range(w0, w1 + 1):
            sched[w].append((len(insts), ch))
            insts.append((ch, w))
    NINST = len(insts)
    S6_c = np.zeros((C, NINST, P, 6 * WA), f32)
    for q in range(C):
        for idx, (ch, w) in enumerate(insts):
            rows = np.arange(ch * P, ch * P + P)
            wn = awin_c[q, rows, 0]
            sl = awin_c[q, rows, 1]
            sel = np.flatnonzero(wn == w)
            if len(sel) == 0:
                continue
            for r in range(6):
                S6_c[q, idx, sel, r * WA + sl[sel]] = \
                    e_rbf_c[q][rows[sel], r]

    # ---- parameter packing ----
    def g(name):
        return np.asarray(inputs[name], f32)

    params = dict(
        emb_rbf_w=g("emb_rbf_w"), emb_w3=emb_w[2 * EMB:],
        iw_ji=g("iw_ji"), iw_kj=g("iw_kj"), iw_down=g("iw_down"),
        iw_up=g("iw_up"), iw_final=g("iw_final"), iw_res=g("iw_res"),
        iw_rbf1=g("iw_rbf1"), iw_rbf2=g("iw_rbf2"),
        ow_dense=g("ow_dense"),
        ow_out=np.concatenate([g("ow_out"), np.zeros_like(g("ow_out"))], axis=-1),
        ow_rbf_t=np.transpose(g("ow_rbf"), (0, 1, 3, 2)).copy(),  # [3,5,128,6]
    )

    meta = dict(
        cfg=cfg, C=C, APC=APC, NWIN=NWIN, APAD=APAD, EPAD=EPAD,
        NSW=NSW, BMAX=BMAX, SLOTS=SLOTS, NT=NT, SWT=SWT,
        NINST=NINST, insts=insts, sched=sched,
        wstart=wstart, nchunk=nchunk,
        win_of_atom=win_of_atom, slot_of_atom=slot_of_atom,
        l_of_old=l_of_old, q_of_old=q_of_old,
    )
    percore = dict(
        e_rbf_fm=np.ascontiguousarray(np.transpose(e_rbf_c, (0, 2, 1))),
        hja_fm=np.ascontiguousarray(np.transpose(hja_c, (0, 2, 1))),
        gidx=gidx_w_c, sbfp=sbfp_c, tgt=tgt_c,
        s6=S6_c.astype(ml_dtypes.bfloat16),
    )
    return meta, percore, params


# ============================ numpy emulator ============================

def emulate_all(meta, pc, params):
    """Full 8-core emulation incl. exchange; returns [3, C, APAD] atomwise."""
    f32 = np.float32
    bf16 = ml_dtypes.bfloat16
    C, EPAD, SLOTS, BMAX = meta["C"], meta["EPAD"], meta["SLOTS"], meta["BMAX"]
    NWIN, NSW, NT, SWT = meta["NWIN"], meta["NSW"], meta["NT"], meta["SWT"]

    def swish(x):
        return (x / (1 + np.exp(-x))).astype(f32)

    e_rbf = [pc["e_rbf_fm"][q].T for q in range(C)]
    m = []
    for q in range(C):
        rbf_e = swish(e_rbf[q] @ params["emb_rbf_w"])
        m.append(swish(pc["hja_fm"][q].T + rbf_e @ params["emb_w3"]))

    gidx = [pc["gidx"][q][:16].T.reshape(-1)[:SLOTS].astype(np.int64)
            for q in range(C)]

    def out_level(q, m_, o, atomwise):
        Q = np.zeros((NWIN, EMB, 6 * WA), f32)
        for w, lst in enumerate(meta["sched"]):
            for (ii, ch) in lst:
                Q[w] += m_[ch * P:(ch + 1) * P].T @ pc["s6"][q][ii]
        for k in range(N_KEYS):
            wk = params["ow_rbf_t"][k, o]
            acc = np.zeros((EMB, NWIN * WA), f32)
            for r in range(6):
                acc += wk[:, r:r + 1] * np.transpose(
                    Q[:, :, r * WA:(r + 1) * WA], (1, 0, 2)).reshape(EMB, -1)
            x = acc
            for l in range(3):
                x = swish(params["ow_dense"][k, o, l].T @ x)
            atomwise[k][q] += params["ow_out"][k, o][:, 0] @ x

    atomwise = [np.zeros((C, meta["APAD"]), f32) for _ in range(N_KEYS)]
    for q in range(C):
        out_level(q, m[q], 0, atomwise)

    for c in range(N_CONV):
        x_ji, send = [], []
        for q in range(C):
            rbf_p = (e_rbf[q] @ params["iw_rbf1"][c]) @ params["iw_rbf2"][c]
            x_ji.append(swish(m[q] @ params["iw_ji"][c]))
            xkj = swish(m[q] @ params["iw_kj"][c])
            xke = swish((xkj * rbf_p) @ params["iw_down"][c]).astype(bf16)
            send.append(xke[gidx[q]])                     # [SLOTS, 64] bf16
        for q in range(C):
            # alltoall: recv[s*BMAX + j] = send[s][q*BMAX + j]
            recv = np.concatenate(
                [send[s][q * BMAX:(q + 1) * BMAX] for s in range(C)])
            agg = np.zeros((EPAD, INT_DIM), f32)
            tgtq = np.asarray(pc["tgt"][q], f32)          # [NT, BSEG, 32]
            for t in range(NT):
                for s in range(C):
                    for swl in range(SWT):
                        r0 = s * BMAX + (t * SWT + swl) * BSEG
                        rows = recv[r0:r0 + BSEG]
                        ri = (t * SWT * C + s * SWT + swl) * BSEG
                        sp = pc["sbfp"][q][c, ri:ri + BSEG]
                        prod = (rows * sp).astype(bf16).astype(f32)
                        tg = tgtq[t, :, s * SWT + swl].astype(np.int64)
                        valid = tg < SWE
                        e0 = t * TILE + swl * SWE
                        np.add.at(agg, e0 + tg[valid], prod[valid])
            hh = x_ji[q] + swish(agg @ params["iw_up"][c])
            zz = swish(swish(hh @ params["iw_res"][c, 0, 0])
                       @ params["iw_res"][c, 0, 1])
            hh = hh + zz
            m[q] = swish(hh @ params["iw_final"][c]) + m[q]
            for r in (1, 2):
                zz = swish(swish(m[q] @ params["iw_res"][c, r, 0])
                           @ params["iw_res"][c, r, 1])
                m[q] = m[q] + zz
            out_level(q, m[q], c + 1, atomwise)
    return atomwise


def host_finalize(meta, atomwise_list, cfg):
    """atomwise_list: [C] of [3, APAD] -> [NG, 2] adiabatic energies."""
    NA, NG, APM = cfg["NA"], cfg["NG"], cfg["APM"]
    C, APC = meta["C"], meta["APC"]
    win_of_atom, slot_of_atom = meta["win_of_atom"], meta["slot_of_atom"]
    E = np.zeros((N_KEYS, NG), np.float64)
    a = np.arange(NA)
    padpos = win_of_atom * WA + slot_of_atom
    mol = a // APM
    for k in range(N_KEYS):
        vals = np.stack([atomwise_list[q][k] for q in range(C)])  # [C, APAD]
        atom_e = vals[a // APC, padpos]
        np.add.at(E[k], mol, atom_e.astype(np.float64))
    d0, d1, lam = E[0], E[1], E[2]
    tr = 0.5 * (d0 + d1)
    rad = np.sqrt((0.5 * (d0 - d1)) ** 2 + lam * lam)
    out = np.stack([tr - rad, tr + rad], axis=-1).astype(np.float32)
    return out


# ============================ Bass program ============================

def build_program(meta):
    import concourse.bacc as bacc
    import concourse.bass as bass
    import concourse.mybir as mybir
    import concourse.tile as tile
    from concourse.masks import make_identity

    f32 = mybir.dt.float32
    fr = mybir.dt.float32r
    bf16 = mybir.dt.bfloat16
    i16 = mybir.dt.int16
    i32 = mybir.dt.int32
    SILU = mybir.ActivationFunctionType.Silu
    ISEQ = mybir.AluOpType.is_equal
    EPAD, SLOTS, BMAX = meta["EPAD"], meta["SLOTS"], meta["BMAX"]
    NWIN, APAD, NINST = meta["NWIN"], meta["APAD"], meta["NINST"]
    NSW, NT, SWT = meta["NSW"], meta["NT"], meta["SWT"]
    C = meta["C"]
    sched = meta["sched"]
    NMT = EPAD // TILE
    XW = 2 * INT_DIM            # padded bf16 row width of x_kj_e
    NSEG = C * SWT              # recv segments per tile

    nc = bacc.Bacc("TRN2", target_bir_lowering=False, debug=False,
                   num_devices=C)

    # ---- I/O (f32r tensors carry plain fp32 bits) ----
    e_rbf_t = nc.dram_tensor("e_rbf_fm", [N_RBF, EPAD], fr, kind="ExternalInput")
    hja_t = nc.dram_tensor("hja_fm", [EMB, EPAD], fr, kind="ExternalInput")
    gidx_t = nc.dram_tensor("gidx", [P, SLOTS // 16], i16, kind="ExternalInput")
    sbfp_t = nc.dram_tensor("sbfp", [N_CONV, SLOTS, INT_DIM], bf16,
                            kind="ExternalInput")
    tgt_t = nc.dram_tensor("tgt", [NT, BSEG, NSEG], f32, kind="ExternalInput")
    s6_t = nc.dram_tensor("s6", [NINST, P, 6 * WA], bf16, kind="ExternalInput")
    p_emb_rbf = nc.dram_tensor("emb_rbf_w", [N_RBF, EMB], fr, kind="ExternalInput")
    p_emb_w3 = nc.dram_tensor("emb_w3", [EMB, EMB], fr, kind="ExternalInput")
    p_ji = nc.dram_tensor("iw_ji", [N_CONV, EMB, EMB], fr, kind="ExternalInput")
    p_kj = nc.dram_tensor("iw_kj", [N_CONV, EMB, EMB], fr, kind="ExternalInput")
    p_down = nc.dram_tensor("iw_down", [N_CONV, EMB, INT_DIM], fr,
                            kind="ExternalInput")
    p_up = nc.dram_tensor("iw_up", [N_CONV, INT_DIM, EMB], fr,
                          kind="ExternalInput")
    p_final = nc.dram_tensor("iw_final", [N_CONV, EMB, EMB], fr,
                             kind="ExternalInput")
    p_res = nc.dram_tensor("iw_res", [N_CONV, 3, 2, EMB, EMB], fr,
                           kind="ExternalInput")
    p_rbf1 = nc.dram_tensor("iw_rbf1", [N_CONV, N_RBF, BEMB], fr,
                            kind="ExternalInput")
    p_rbf2 = nc.dram_tensor("iw_rbf2", [N_CONV, BEMB, EMB], fr,
                            kind="ExternalInput")
    p_owd = nc.dram_tensor("ow_dense", [N_KEYS, NLEVEL, 3, EMB, EMB], fr,
                           kind="ExternalInput")
    p_owo = nc.dram_tensor("ow_out", [N_KEYS, NLEVEL, EMB, 2], fr,
                           kind="ExternalInput")
    p_owr = nc.dram_tensor("ow_rbf_t", [N_KEYS, NLEVEL, EMB, N_RBF], f32,
                           kind="ExternalInput")
    atw_t = nc.dram_tensor("atomwise", [N_KEYS, APAD], f32,
                           kind="ExternalOutput")

    with tile.TileContext(nc) as tc:
        with (
            tc.tile_pool(name="wk", bufs=3) as wk,
            tc.tile_pool(name="hhp", bufs=1) as hhp,
            tc.tile_pool(name="cst", bufs=1) as cst,
            tc.tile_pool(name="par", bufs=2) as par,
            tc.tile_pool(name="gth", bufs=2) as gth,
            tc.tile_pool(name="prp", bufs=2) as prp,
            tc.tile_pool(name="indp", bufs=2) as indp,
            tc.tile_pool(name="idxp", bufs=2) as idxp,
            tc.tile_pool(name="s6p", bufs=2) as s6p,
            tc.tile_pool(name="mtp", bufs=2) as mtp,
            tc.tile_pool(name="ps", bufs=2, space="PSUM") as ps,
            tc.tile_pool(name="ps_s", bufs=2, space="PSUM") as ps_s,
            tc.tile_pool(name="ps_t", bufs=2, space="PSUM") as ps_t,
            tc.tile_pool(name="dram", bufs=1, space="DRAM") as dram,
        ):
            identf = cst.tile([P, P], f32, tag="identf")
            make_identity(nc, identf[:])
            ident = cst.tile([P, P], fr, tag="ident")
            nc.vector.tensor_copy(ident[:], identf[:])
            m_sb = cst.tile([EMB, EPAD], fr, tag="m")
            Q_sb = cst.tile([EMB, NWIN * 6 * WA], f32, tag="Q")
            atw_sb = cst.tile([P, APAD + (APAD & 1)], f32, tag="atw")
            nc.vector.memset(atw_sb[:], 0.0)
            zer_sb = cst.tile([P, 2 * INT_DIM], f32, tag="zer")
            nc.vector.memset(zer_sb[:], 0.0)
            iota_i = cst.tile([P, SWE], i32, tag="iotai")
            nc.gpsimd.iota(iota_i[:], pattern=[[1, SWE]], base=0,
                           channel_multiplier=0)
            iota_bf = cst.tile([P, SWE], bf16, tag="iotab")
            nc.vector.tensor_copy(iota_bf[:], iota_i[:])

            xkje_d = dram.tile([EPAD, XW], bf16, tag="xkje")
            xji_d = dram.tile([EMB, EPAD], bf16, tag="xji")
            a2a_in = dram.tile([SLOTS, INT_DIM], bf16, tag="a2ai")
            a2a_out = dram.tile([SLOTS, INT_DIM], bf16, tag="a2ao")

            def act(dst, src, func=SILU):
                nc.scalar.activation(dst, src, func)

            # -------- embedding --------
            embw_sb = par.tile([N_RBF, EMB], fr, tag="p0")
            nc.sync.dma_start(embw_sb[:], p_emb_rbf[:])
            w3_sb = par.tile([EMB, EMB], fr, tag="p1")
            nc.sync.dma_start(w3_sb[:], p_emb_w3[:])
            for t in range(0 if os.environ.get("K_SKIP_EMB", "0") == "1" else NMT):
                sl = slice(t * TILE, (t + 1) * TILE)
                erbf = wk.tile([N_RBF, TILE], fr, tag="w")
                nc.sync.dma_start(erbf[:], e_rbf_t[:, sl])
                psr = ps.tile([P, TILE], f32, space="PSUM", tag="big")
                for h in range(2):
                    hsl = slice(h * 512, (h + 1) * 512)
                    nc.tensor.matmul(psr[:, hsl], lhsT=embw_sb[:],
                                     rhs=erbf[:, hsl], start=True, stop=True)
                rbfe = wk.tile([P, TILE], fr, tag="w")
                act(rbfe[:], psr[:])
                psm = ps.tile([P, TILE], f32, space="PSUM", tag="big")
                hj = wk.tile([P, TILE], fr, tag="w")
                nc.sync.dma_start(hj[:], hja_t[:, sl])
                for h in range(2):
                    hsl = slice(h * 512, (h + 1) * 512)
                    nc.tensor.matmul(psm[:, hsl], lhsT=w3_sb[:],
                                     rhs=rbfe[:, hsl], start=True, stop=False)
                    nc.tensor.matmul(psm[:, hsl], lhsT=ident[:],
                                     rhs=hj[:, hsl], start=False, stop=True)
                act(m_sb[:, sl], psm[:])

            # -------- out-block --------
            def out_block(lvl):
                for w in range(NWIN):
                    psq = ps_s.tile([P, 6 * WA], f32, space="PSUM", tag="small")
                    lst = sched[w]
                    blocks = [lst[i:i + 2] for i in range(0, len(lst), 2)]
                    n = 0
                    for blk in blocks:
                        i0 = blk[0][0]
                        s6i = s6p.tile([P, 2, 6 * WA], bf16, tag="s6")
                        nc.sync.dma_start(s6i[:, :len(blk), :],
                                          s6_t[i0:i0 + len(blk)]
                                          .rearrange("i p x -> p i x"))
                        for j, (ii, ch) in enumerate(blk):
                            pst = ps_t.tile([P, P], fr, space="PSUM", tag="t")
                            nc.tensor.transpose(
                                pst[:], m_sb[:, ch * P:(ch + 1) * P], ident[:])
                            mt = mtp.tile([P, P], bf16, tag="mt")
                            nc.vector.tensor_copy(mt[:], pst[:])
                            nc.tensor.matmul(psq[:], lhsT=mt[:],
                                             rhs=s6i[:, j, :],
                                             start=(n == 0),
                                             stop=(n == len(lst) - 1))
                            n += 1
                    nc.vector.tensor_copy(
                        Q_sb[:, w * 6 * WA:(w + 1) * 6 * WA], psq[:])
                APE = APAD + (APAD & 1)
                chunks = [(o, min(512, APE - o)) for o in range(0, APE, 512)]
                for k in range(N_KEYS):
                    wko = par.tile([EMB, N_RBF], f32, tag="p2")
                    nc.sync.dma_start(wko[:], p_owr[k, lvl])
                    hh = hhp.tile([EMB, APE], fr, tag="hh")
                    if APE != APAD:
                        nc.vector.tensor_copy(hh[:, APAD:APE],
                                              zer_sb[:, :APE - APAD])
                    qv = Q_sb[:].rearrange("p (w rs) -> p w rs", rs=6 * WA)
                    hv = hh[:, :APAD].rearrange("p (w a) -> p w a", a=WA)
                    for r in range(6):
                        qr = qv[:, :, r * WA:(r + 1) * WA]
                        if r == 0:
                            nc.vector.tensor_scalar_mul(hv, qr, wko[:, 0:1])
                        else:
                            nc.vector.scalar_tensor_tensor(
                                hv, qr, wko[:, r:r + 1], hv,
                                op0=mybir.AluOpType.mult,
                                op1=mybir.AluOpType.add)
                    for l in range(3):
                        wd = par.tile([EMB, EMB], fr, tag="p3")
                        nc.sync.dma_start(wd[:], p_owd[k, lvl, l])
                        for (o, w_) in chunks:
                            psd = ps_s.tile([P, 512], f32, space="PSUM",
                                            tag="small")
                            nc.tensor.matmul(psd[:, :w_], lhsT=wd[:],
                                             rhs=hh[:, o:o + w_],
                                             start=True, stop=True)
                            act(hh[:, o:o + w_], psd[:, :w_])
                    wo = par.tile([EMB, 2], fr, tag="p2b")
                    nc.sync.dma_start(wo[:], p_owo[k, lvl])
                    for (o, w_) in chunks:
                        pse = ps_s.tile([2, 512], f32, space="PSUM", tag="small")
                        nc.tensor.matmul(pse[:, :w_], lhsT=wo[:],
                                         rhs=hh[:, o:o + w_],
                                         start=True, stop=True)
                        av = atw_sb[32 * k:32 * k + 1, o:o + w_]
                        nc.vector.tensor_add(av, av, pse[0:1, :w_])

            skip_out = os.environ.get("K_SKIP_OUT", "0") == "1"
            skip_ang = os.environ.get("K_SKIP_ANG", "0") == "1"
            skip_stgA = os.environ.get("K_SKIP_STGA", "0") == "1"
            skip_stgD = os.environ.get("K_SKIP_STGD", "0") == "1"
            # -------- conv loop --------
            for c in range(N_CONV):
                wji = par.tile([EMB, EMB], fr, tag="c0")
                nc.sync.dma_start(wji[:], p_ji[c])
                wkj = par.tile([EMB, EMB], fr, tag="c1")
                nc.sync.dma_start(wkj[:], p_kj[c])
                wdn = par.tile([EMB, INT_DIM], fr, tag="c2")
                nc.sync.dma_start(wdn[:], p_down[c])
                wr1 = par.tile([N_RBF, BEMB], fr, tag="c3")
                nc.sync.dma_start(wr1[:], p_rbf1[c])
                wr2 = par.tile([BEMB, EMB], fr, tag="c4")
                nc.sync.dma_start(wr2[:], p_rbf2[c])

                # stage A: x_ji spill, x_kj_e (bf16-padded rows) to HBM
                for t in range(0 if skip_stgA else NMT):
                    sl = slice(t * TILE, (t + 1) * TILE)
                    psj = ps.tile([P, TILE], f32, space="PSUM", tag="big")
                    for h in range(2):
                        hsl = slice(h * 512, (h + 1) * 512)
                        nc.tensor.matmul(
                            psj[:, hsl], lhsT=wji[:],
                            rhs=m_sb[:, t * TILE + h * 512:
                                     t * TILE + (h + 1) * 512],
                            start=True, stop=True)
                    xji = wk.tile([P, TILE], bf16, tag="w")
                    act(xji[:], psj[:])
                    nc.sync.dma_start(xji_d[:, sl], xji[:])

                    psk = ps.tile([P, TILE], f32, space="PSUM", tag="big")
                    for h in range(2):
                        hsl = slice(h * 512, (h + 1) * 512)
                        nc.tensor.matmul(
                            psk[:, hsl], lhsT=wkj[:],
                            rhs=m_sb[:, t * TILE + h * 512:
                                     t * TILE + (h + 1) * 512],
                            start=True, stop=True)
                    xkj = wk.tile([P, TILE], fr, tag="w")
                    act(xkj[:], psk[:])

                    t1 = wk.tile([BEMB, TILE], fr, tag="w")
                    erbf = wk.tile([N_RBF, TILE], fr, tag="w")
                    nc.sync.dma_start(erbf[:], e_rbf_t[:, sl])
                    for h in range(2):
                        hsl = slice(h * 512, (h + 1) * 512)
                        ps1 = ps_s.tile([BEMB, 512], f32, space="PSUM",
                                        tag="small")
                        nc.tensor.matmul(ps1[:], lhsT=wr1[:], rhs=erbf[:, hsl],
                                         start=True, stop=True)
                        nc.vector.tensor_copy(t1[:, hsl], ps1[:])
                    psp = ps.tile([P, TILE], f32, space="PSUM", tag="big")
                    for h in range(2):
                        hsl = slice(h * 512, (h + 1) * 512)
                        nc.tensor.matmul(psp[:, hsl], lhsT=wr2[:],
                                         rhs=t1[:, hsl], start=True, stop=True)
                    xm = wk.tile([P, TILE], fr, tag="w")
                    nc.vector.tensor_mul(xm[:], xkj[:], psp[:])

                    pse4 = ps_s.tile([P, 512], f32, space="PSUM", tag="small")
                    for i in range(8):
                        nc.tensor.matmul(pse4[:, i * 64:(i + 1) * 64],
                                         lhsT=xm[:, i * P:(i + 1) * P],
                                         rhs=wdn[:], start=True, stop=True)
                    xke = wk.tile([P, 512], bf16, tag="w")
                    act(xke[:], pse4[:])
                    nc.sync.dma_start(
                        xkje_d[:].rearrange("(b p) x -> p b x", p=P)
                        [:, t * 8:(t + 1) * 8, :INT_DIM],
                        xke[:].rearrange("p (b e) -> p b e", e=INT_DIM))

                # out-block for level c overlaps the gather + collective
                if not skip_out:
                    out_block(c)

                # stage B: gather raw x_kj_e rows into a2a slots
                for g in range(0 if skip_ang else SLOTS // GCH):
                    gi = idxp.tile([P, GCH // 16], i16, tag="gi")
                    nc.sync.dma_start(
                        gi[:], gidx_t[:, g * GCH // 16:(g + 1) * GCH // 16])
                    gb = gth.tile([P, GCH // P, XW], bf16, tag="gb")
                    nc.gpsimd.dma_gather(
                        gb[:], xkje_d[:], gi[:], num_idxs=GCH,
                        num_idxs_reg=GCH, elem_size=XW,
                        single_packet=False)
                    nc.sync.dma_start(
                        a2a_in[g * GCH:(g + 1) * GCH, :]
                        .rearrange("(b p) e -> p b e", p=P),
                        gb[:, :, :INT_DIM])

                if not skip_ang:
                    nc.gpsimd.collective_compute(
                        "AllToAll", mybir.AluOpType.bypass,
                        replica_groups=[list(range(C))],
                        ins=[a2a_in[:].opt()],
                        outs=[a2a_out[:].opt()])

                # stage D: receiver-side products + indicator-matmul
                # aggregation fused with the rest of the conv
                wup = par.tile([INT_DIM, EMB], fr, tag="c0")
                nc.sync.dma_start(wup[:], p_up[c])
                wfi = par.tile([EMB, EMB], fr, tag="c1")
                nc.sync.dma_start(wfi[:], p_final[c])
                wres = []
                for r in range(3):
                    for s_ in range(2):
                        wres_t = par.tile([EMB, EMB], fr, tag=f"r{r}{s_}")
                        wres.append(wres_t)
                for r in range(3):
                    for s_ in range(2):
                        nc.sync.dma_start(wres[r * 2 + s_][:], p_res[c, r, s_])

                for t in range(0 if skip_stgD else NMT):
                    sl = slice(t * TILE, (t + 1) * TILE)
                    afm = wk.tile([INT_DIM, TILE], fr, tag="w")
                    if skip_ang:
                        nc.vector.memset(afm[:], 0.0)
                    else:
                        prod = prp.tile([P, C, SWT, INT_DIM], bf16, tag="pr")
                        for s in range(C):
                            r0 = s * BMAX + t * (SWT * BSEG)
                            nc.sync.dma_start(
                                prod[:, s, :, :],
                                a2a_out[r0:r0 + SWT * BSEG, :]
                                .rearrange("(g p) e -> p g e", p=P))
                        sbf = prp.tile([P, C, SWT, INT_DIM], bf16, tag="sb")
                        nc.sync.dma_start(
                            sbf[:],
                            sbfp_t[c, t * (SWT * BSEG * C):
                                   (t + 1) * (SWT * BSEG * C), :]
                            .rearrange("(g p) e -> p g e", p=P)
                            .rearrange("p (s w) e -> p s w e", w=SWT))
                        nc.vector.tensor_mul(prod[:], prod[:], sbf[:])
                        tgs = idxp.tile([P, NSEG], f32, tag="tg")
                        nc.sync.dma_start(tgs[:], tgt_t[t])
                        psg = ps.tile([P, TILE], f32, space="PSUM", tag="big")
                        for swl in range(SWT):
                            ind = indp.tile([P, C, SWE], bf16, tag="in")
                            for s in range(C):
                                nc.vector.tensor_scalar(
                                    ind[:, s, :], iota_bf[:],
                                    tgs[:, s * SWT + swl:s * SWT + swl + 1],
                                    None, op0=ISEQ)
                            for s in range(C):
                                nc.tensor.matmul(
                                    psg[:INT_DIM, swl * SWE:(swl + 1) * SWE],
                                    lhsT=prod[:, s, swl, :], rhs=ind[:, s, :],
                                    start=(s == 0), stop=(s == C - 1))
                        nc.vector.tensor_copy(afm[:], psg[:INT_DIM, :])

                    psu = ps.tile([P, TILE], f32, space="PSUM", tag="big")
                    for h in range(2):
                        hsl = slice(h * 512, (h + 1) * 512)
                        nc.tensor.matmul(psu[:, hsl], lhsT=wup[:],
                                         rhs=afm[:, hsl], start=True, stop=True)
                    hh = wk.tile([P, TILE], fr, tag="w")
                    act(hh[:], psu[:])
                    xji = wk.tile([P, TILE], bf16, tag="w")
                    nc.sync.dma_start(xji[:], xji_d[:, sl])
                    nc.vector.tensor_add(hh[:], hh[:], xji[:])

                    def res_pair(vin, w0, w1):
                        psa = ps.tile([P, TILE], f32, space="PSUM", tag="big")
                        for h in range(2):
                            hsl = slice(h * 512, (h + 1) * 512)
                            nc.tensor.matmul(psa[:, hsl], lhsT=w0[:],
                                             rhs=vin[:, hsl], start=True,
                                             stop=True)
                        zz = wk.tile([P, TILE], fr, tag="w")
                        act(zz[:], psa[:])
                        psb = ps.tile([P, TILE], f32, space="PSUM", tag="big")
                        for h in range(2):
                            hsl = slice(h * 512, (h + 1) * 512)
                            nc.tensor.matmul(psb[:, hsl], lhsT=w1[:],
                                             rhs=zz[:, hsl], start=True,
                                             stop=True)
                        act(zz[:], psb[:])
                        nc.vector.tensor_add(vin[:], vin[:], zz[:])

                    res_pair(hh, wres[0], wres[1])
                    psf = ps.tile([P, TILE], f32, space="PSUM", tag="big")
                    for h in range(2):
                        hsl = slice(h * 512, (h + 1) * 512)
                        nc.tensor.matmul(psf[:, hsl], lhsT=wfi[:],
                                         rhs=hh[:, hsl], start=True, stop=True)
                    fz = wk.tile([P, TILE], fr, tag="w")
                    act(fz[:], psf[:])
                    mview = m_sb[:, sl]
                    nc.vector.tensor_add(mview, mview, fz[:])
                    res_pair(mview, wres[2], wres[3])
                    res_pair(mview, wres[4], wres[5])

            if not skip_out:
                out_block(N_CONV)

            for k in range(N_KEYS):
                nc.sync.dma_start(atw_t[k:k + 1, :],
                                  atw_sb[32 * k:32 * k + 1, :APAD])

    nc.compile()
    return nc


# ============================ runner ============================

_CACHE = {}
_LAST_RESULT = None


def _in_map(percore, params, q):
    im = dict(
        e_rbf_fm=percore["e_rbf_fm"][q], hja_fm=percore["hja_fm"][q],
        gidx=percore["gidx"][q], sbfp=percore["sbfp"][q],
        tgt=percore["tgt"][q], s6=percore["s6"][q],
    )
    im.update(params)
    return im


def _run(meta, percore, params, cfg):
    from concourse.bass_utils import run_bass_kernel_spmd
    key = (meta["EPAD"], meta["SLOTS"], meta["BMAX"], meta["NINST"])
    if key not in _CACHE:
        _CACHE[key] = build_program(meta)
    nc = _CACHE[key]
    C = meta["C"]
    in_maps = [_in_map(percore, params, q) for q in range(C)]
    trace = os.environ.get("KERNEL_TRACE", "0") == "1"
    res = run_bass_kernel_spmd(nc, in_maps, core_ids=list(range(C)),
                               trace=trace)
    global _LAST_RESULT
    _LAST_RESULT = res
    return [r["atomwise"] for r in res.results]


def kernel(**inputs):
    cfg = FULL_CFG
    meta, percore, params = host_prep(inputs, cfg)
    atw = _run(meta, percore, params, cfg)
    return host_finalize(meta, atw, cfg)
